# revision 11
# baseline (speedup 1.0000x reference)
"""HDC generic encoder kernel for 8 Trainium2 NeuronCores.

out[b,d] = sum_{w=0..56} K[w,d] * prod_{j=0..6} enc0[b, w+1+j, (d-(6-j)) mod D]

enc0[b,p,:] is a +/-1 table row selected by level-quantizing x[b,0,p].
Sharding: pure data parallel over batch, 8 batches per core.

Device pipeline per core:
1. Table is pre-blocked in HBM as [VROWS, 16*632]: tbl[r, k*632+e] =
   row_r[(k*625 + e - 6) mod 10000], so each gathered row already holds
   the 16 d-blocks with their 7-element halos materialized.
2. 4 indirect row-gathers R_i[128, 10112] for batches {2i, 2i+1}
   (partition p = h*64 + pos -> batch 2i+h).
3. Redistribution to compute layout G[q = b*16 + blk][s*632 + e] via a
   DRAM bounce: R_i -> dram scratch in G-order (src reads 128 spread
   partitions, dst scatters 1264B runs in DRAM - both fast DMA
   patterns), then a contiguous 80KB-per-partition readback into G.
   This replaces the direct SBUF->SBUF transpose whose 64 packets per
   DMA all targeted ONE dst partition (one port) and serialized at
   ~737 ns/packet (~380 us total in the previous version).
4. Window products via a 5-op tensor_tensor tree (even offsets
   everywhere except the final even*odd combine which runs at 1x):
     E1[p,e]  = G[p*SEG+e]   * G[(p+2)*SEG+e+2]     (factor pairs j, j+2)
     E2[w,d'] = E1[w+1,d']   * E1[w+5,d'+4]         (factors 0,2,4,6)
     O3[w,e]  = E1[w+2,e]    * G[(w+6)*SEG+e+4]     (factors 1,3,5; e=d'+1)
     PROD     = E2[w,d']     * O3[w,d'+1]
     BD       = PROD * keys
5. log-tree adds over w (exact in bf16: +-1 sums <= 57), f32 accumulation
   across w-chunks, one DMA of the [128, 625] accumulator to out [8, 10000].
"""

import numpy as np

import concourse.bacc as bacc
import concourse.bass as bass
import concourse.mybir as mybir
from concourse.bass_utils import run_bass_kernel_spmd
from concourse.tile import TileContext

B, T, F, D = 64, 4, 64, 10000
NGRAMS = 7
W = F - NGRAMS  # 57 windows
NCORES = 8
BPC = B // NCORES  # 8 batches per core
MROWS, HROWS = 3000, 200
VROWS = MROWS + HROWS

NBLK = 16
BLKW = D // NBLK  # 625
SEGW = 632  # valid elems per segment (625 + 6 halo + 1)
SEG = 640  # segment stride in G (640 not 632: 632 measured ~27% slower DVE)
ET = NBLK * SEGW  # pre-blocked table row width: 10112
KW = 626  # keys/intermediate per-window stride (even)
SGRP = 8  # segments per bounce/readback group (pipelining granularity)

W_E1 = 630
W_E2 = 626
W_O3 = 626
W_PR = 625

CHUNKS = [(0, 8), (8, 8), (16, 8), (24, 8), (32, 8), (40, 8), (48, 9)]

_CACHE = {}


def _build_nc():
    nc = bacc.Bacc(None)
    tbl = nc.dram_tensor("tbl", [VROWS, ET], mybir.dt.bfloat16, kind="ExternalInput")
    keys2 = nc.dram_tensor(
        "keys2", [128, W * KW], mybir.dt.bfloat16, kind="ExternalInput"
    )
    goff = nc.dram_tensor("goff", [128, 4], mybir.dt.int32, kind="ExternalInput")
    out = nc.dram_tensor("out", [BPC, D], mybir.dt.float32, kind="ExternalOutput")
    out_r = out.rearrange("b (q d) -> (b q) d", d=BLKW)  # [128, 625]

    gb = nc.dram_tensor("gbounce", [128, F * SEGW], mybir.dt.bfloat16, kind="Internal")
    # gbv[b, s, k, e]: G-order view of the bounce buffer (row q = 16b + k)
    gbv = gb.rearrange("(b k) (s e) -> b s k e", k=NBLK, e=SEGW)

    with TileContext(nc) as tc:
        with (
            tc.tile_pool(name="big", bufs=1) as bpool,
            tc.tile_pool(name="rp", bufs=2) as rpool,
            tc.tile_pool(name="work", bufs=1) as wpool,
            tc.tile_pool(name="keysp", bufs=2) as kpool,
        ):
            goff_t = bpool.tile([128, 4], mybir.dt.int32, tag="goff")
            nc.sync.dma_start(out=goff_t[:, :], in_=goff[:, :])

            g = bpool.tile([128, F * SEG], mybir.dt.bfloat16, tag="G")
            g3 = g[:, :].rearrange("p (s e) -> p s e", e=SEG)
            gb3 = gb.rearrange("q (s e) -> q s e", e=SEGW)
            # bounce + readback in segment groups so compute can start as soon
            # as the first groups of all tiles have landed in G
            ngrp = F // SGRP
            for i in range(4):
                r = rpool.tile([128, ET], mybir.dt.bfloat16, tag="R", name=f"R{i}")
                nc.gpsimd.indirect_dma_start(
                    out=r[:, :],
                    out_offset=None,
                    in_=tbl[:, :],
                    in_offset=bass.IndirectOffsetOnAxis(ap=goff_t[:, i : i + 1], axis=0),
                )
                r3 = r[:, :].rearrange("p (k e) -> p k e", e=SEGW)
                for t in range(ngrp):
                    s0 = t * SGRP
                    for h in range(2):
                        nc.sync.dma_start(
                            out=gbv[2 * i + h, s0 : s0 + SGRP, :, :],
                            in_=r3[64 * h + s0 : 64 * h + s0 + SGRP, :, :],
                        )
                    nc.sync.dma_start(
                        out=g3[32 * i : 32 * (i + 1), s0 : s0 + SGRP, 0:SEGW],
                        in_=gb3[32 * i : 32 * (i + 1), s0 : s0 + SGRP, :],
                    )

            acc = bpool.tile([128, KW], mybir.dt.float32, tag="acc")
            nc.vector.memset(acc[:, :], 0.0)

            def rv(tile, stride, cnt, off, width):
                """Strided-row view: rows of `width` elems at `off + i*stride`."""
                base = (off // stride) * stride
                o2 = off - base
                v = tile[:, base : base + cnt * stride].rearrange(
                    "p (s k) -> p s k", k=stride
                )
                return v[:, :, o2 : o2 + width]

            for w0, wc in CHUNKS:
                kc = kpool.tile([128, wc * KW], mybir.dt.bfloat16, tag="kc")
                nc.sync.dma_start(out=kc[:, :], in_=keys2[:, w0 * KW : (w0 + wc) * KW])

                ne1 = wc + 4
                e1 = wpool.tile([128, ne1 * W_E1], mybir.dt.bfloat16, tag="e1")
                nc.vector.tensor_mul(
                    rv(e1, W_E1, ne1, 0, W_E1),
                    rv(g, SEG, ne1, (w0 + 1) * SEG, W_E1),
                    rv(g, SEG, ne1, (w0 + 3) * SEG + 2, W_E1),
                )
                e2 = wpool.tile([128, wc * KW], mybir.dt.bfloat16, tag="e2")
                nc.vector.tensor_mul(
                    rv(e2, KW, wc, 0, W_E2),
                    rv(e1, W_E1, wc, 0, W_E2),
                    rv(e1, W_E1, wc, 4 * W_E1 + 4, W_E2),
                )
                o3 = wpool.tile([128, wc * KW], mybir.dt.bfloat16, tag="o3")
                nc.vector.tensor_mul(
                    rv(o3, KW, wc, 0, W_O3),
                    rv(e1, W_E1, wc, W_E1, W_O3),
                    rv(g, SEG, wc, (w0 + 6) * SEG + 4, W_O3),
                )
                pr = wpool.tile([128, wc * KW], mybir.dt.bfloat16, tag="pr")
                nc.vector.tensor_mul(
                    rv(pr, KW, wc, 0, W_PR),
                    rv(e2, KW, wc, 0, W_PR),
                    rv(o3, KW, wc, 1, W_PR),
                )
                bd = wpool.tile([128, wc * KW], mybir.dt.bfloat16, tag="bd")
                nc.vector.tensor_mul(
                    rv(bd, KW, wc, 0, W_PR),
                    rv(pr, KW, wc, 0, W_PR),
                    rv(kc, KW, wc, 0, W_PR),
                )
                n = wc
                while n > 1:
                    m = n // 2
                    nc.vector.tensor_add(
                        rv(bd, KW, m, 0, W_PR),
                        rv(bd, KW, m, 0, W_PR),
                        rv(bd, KW, m, (n - m) * KW, W_PR),
                    )
                    n -= m
                nc.vector.tensor_add(acc[:, 0:W_PR], acc[:, 0:W_PR], bd[:, 0:W_PR])

            nc.sync.dma_start(out=out_r[:, :], in_=acc[:, 0:BLKW])
    nc.compile()
    return nc


def _host_prep(x, keys_weight, motion_table, hr_table):
    import ml_dtypes

    bf16 = ml_dtypes.bfloat16

    x0 = np.asarray(x[:, 0, :], dtype=np.float32)  # [B, F]
    # np.round and jnp.round both use round-half-to-even
    mi = np.round((x0[:, : F - 1] - (-3.0)) / (3.0 - (-3.0)) * (MROWS - 1)).astype(
        np.int32
    )
    mi = np.clip(mi, 0, MROWS - 1)
    hi = np.round((x0[:, F - 1] - 50.0) / (200.0 - 50.0) * (HROWS - 1)).astype(
        np.int32
    )
    hi = np.clip(hi, 0, HROWS - 1) + MROWS
    rows = np.concatenate([mi, hi[:, None]], axis=1).astype(np.int32)  # [B, F]

    if "tbl" not in _CACHE:
        tb = np.concatenate(
            [np.asarray(motion_table), np.asarray(hr_table)], axis=0
        ).astype(bf16)  # [VROWS, D]
        # Pre-blocked with halo: tbl[r, k*632 + e] = tb[r, (k*625 + e - 6) % D]
        col_idx = (
            np.arange(NBLK)[:, None] * BLKW + np.arange(SEGW)[None, :] - 6
        ) % D  # [16, 632]
        _CACHE["tbl"] = np.ascontiguousarray(tb[:, col_idx.reshape(-1)])

        kb = np.asarray(keys_weight)[:W].astype(bf16)  # [57, D]
        karr = np.zeros((NBLK, W, KW), dtype=bf16)
        karr[:, :, :BLKW] = kb.reshape(W, NBLK, BLKW).transpose(1, 0, 2)
        _CACHE["keys2"] = np.tile(karr.reshape(NBLK, W * KW), (BPC, 1))
    tbl = _CACHE["tbl"]
    keys2 = _CACHE["keys2"]

    in_maps = []
    for c in range(NCORES):
        r8 = rows[BPC * c : BPC * (c + 1)]  # [8, F]
        # goff[p, i] = row index for batch 2i + p//64, pos p%64
        gof = r8.reshape(4, 2 * F).T.copy().astype(np.int32)  # [128, 4]
        in_maps.append({"tbl": tbl, "keys2": keys2, "goff": gof})
    return in_maps


def run(inputs, trace=False):
    if "nc" not in _CACHE:
        _CACHE["nc"] = _build_nc()
    nc = _CACHE["nc"]
    in_maps = _host_prep(**inputs)
    res = run_bass_kernel_spmd(nc, in_maps, core_ids=list(range(NCORES)), trace=trace)
    outs = [res.results[c]["out"] for c in range(NCORES)]
    full = np.concatenate(outs, axis=0).astype(np.float32)
    return full, res


def kernel(**inputs) -> np.ndarray:
    full, _ = run(inputs, trace=False)
    return full


# revision 17
# speedup vs baseline: 1.2909x; 1.2909x over previous
"""HDC generic encoder kernel for 8 Trainium2 NeuronCores.

out[b,d] = sum_{w=0..56} K[w,d] * prod_{j=0..6} enc0[b, w+1+j, (d-(6-j)) mod D]

enc0[b,p,:] is a +/-1 table row selected by level-quantizing x[b,0,p].
Sharding: pure data parallel over batch, 8 batches per core.

Device pipeline per core:
1. Table is pre-blocked in HBM as [VROWS, 16*632]: tbl[r, k*632+e] =
   row_r[(k*625 + e - 6) mod 10000], so each gathered row already holds
   the 16 d-blocks with their 7-element halos materialized.
2. 4 indirect row-gathers R_i[128, 10112] for batches {2i, 2i+1}
   (partition p = h*64 + pos -> batch 2i+h).
3. Redistribution to compute layout G[q = b*16 + blk][s*632 + e] via a
   DRAM bounce: R_i -> dram scratch in G-order (src reads 128 spread
   partitions, dst scatters 1264B runs in DRAM - both fast DMA
   patterns), then a contiguous 80KB-per-partition readback into G.
   This replaces the direct SBUF->SBUF transpose whose 64 packets per
   DMA all targeted ONE dst partition (one port) and serialized at
   ~737 ns/packet (~380 us total in the previous version).
4. Window products via a 5-op tensor_tensor tree (even offsets
   everywhere except the final even*odd combine which runs at 1x):
     E1[p,e]  = G[p*SEG+e]   * G[(p+2)*SEG+e+2]     (factor pairs j, j+2)
     E2[w,d'] = E1[w+1,d']   * E1[w+5,d'+4]         (factors 0,2,4,6)
     O3[w,e]  = E1[w+2,e]    * G[(w+6)*SEG+e+4]     (factors 1,3,5; e=d'+1)
     PROD     = E2[w,d']     * O3[w,d'+1]
     BD       = PROD * keys
5. log-tree adds over w (exact in bf16: +-1 sums <= 57), f32 accumulation
   across w-chunks, one DMA of the [128, 625] accumulator to out [8, 10000].
"""

import numpy as np

import concourse.bacc as bacc
import concourse.bass as bass
import concourse.mybir as mybir
from concourse.bass_utils import run_bass_kernel_spmd
from concourse.tile import TileContext

B, T, F, D = 64, 4, 64, 10000
NGRAMS = 7
W = F - NGRAMS  # 57 windows
NCORES = 8
BPC = B // NCORES  # 8 batches per core
MROWS, HROWS = 3000, 200
VROWS = MROWS + HROWS

NBLK = 16
BLKW = D // NBLK  # 625
SEGW = 632  # valid elems per segment (625 + 6 halo + 1)
SEG = 640  # segment stride in G (640 not 632: 632 measured ~27% slower DVE)
ET = NBLK * SEG  # pre-blocked table row width: 10240 (blocks padded to 640)
KW = 626  # keys/intermediate per-window stride (even)
SGRP = 8  # segments per bounce/readback group (pipelining granularity)

W_E1 = 630
W_E2 = 626
W_O3 = 626
W_PR = 625

CHUNKS = [(0, 8), (8, 8), (16, 8), (24, 8), (32, 8), (40, 8), (48, 9)]

_CACHE = {}


def _build_nc():
    nc = bacc.Bacc(None)
    tbl = nc.dram_tensor("tbl", [VROWS, ET], mybir.dt.bfloat16, kind="ExternalInput")
    keys2 = nc.dram_tensor(
        "keys2", [128, W * KW], mybir.dt.bfloat16, kind="ExternalInput"
    )
    goff = nc.dram_tensor("goff", [128, 4], mybir.dt.int32, kind="ExternalInput")
    out = nc.dram_tensor("out", [BPC, D], mybir.dt.float32, kind="ExternalOutput")
    out_r = out.rearrange("b (q d) -> (b q) d", d=BLKW)  # [128, 625]

    gb = nc.dram_tensor("gbounce", [128, F * SEG], mybir.dt.bfloat16, kind="Internal")
    # gbv[b, s, k, e]: G-order view of the bounce buffer (row q = 16b + k).
    # Same 640-stride layout as G so the readback is one contiguous copy.
    gbv = gb.rearrange("(b k) (s e) -> b s k e", k=NBLK, e=SEG)

    with TileContext(nc) as tc:
        with (
            tc.tile_pool(name="big", bufs=1) as bpool,
            tc.tile_pool(name="rp", bufs=2) as rpool,
            tc.tile_pool(name="work", bufs=1) as wpool,
            tc.tile_pool(name="keysp", bufs=2) as kpool,
        ):
            goff_t = bpool.tile([128, 4], mybir.dt.int32, tag="goff")
            nc.sync.dma_start(out=goff_t[:, :], in_=goff[:, :])

            g = bpool.tile([128, F * SEG], mybir.dt.bfloat16, tag="G")
            # bounce on the SP ring (nc.sync), readback on the ACT ring
            # (nc.scalar) so readback semaphore waits never stall the SP
            # sequencer.  Split into two 32-segment halves so compute on the
            # first half overlaps the fill of the second.
            HS = F // 2  # 32 segments per half
            for i in range(4):
                r = rpool.tile([128, ET], mybir.dt.bfloat16, tag="R", name=f"R{i}")
                nc.gpsimd.indirect_dma_start(
                    out=r[:, :],
                    out_offset=None,
                    in_=tbl[:, :],
                    in_offset=bass.IndirectOffsetOnAxis(ap=goff_t[:, i : i + 1], axis=0),
                )
                r3 = r[:, :].rearrange("p (k e) -> p k e", e=SEG)
                for half in range(2):
                    s0 = half * HS
                    for h in range(2):
                        nc.sync.dma_start(
                            out=gbv[2 * i + h, s0 : s0 + HS, :, :],
                            in_=r3[64 * h + s0 : 64 * h + s0 + HS, :, :],
                        )
                    nc.scalar.dma_start(
                        out=g[32 * i : 32 * (i + 1), s0 * SEG : (s0 + HS) * SEG],
                        in_=gb[32 * i : 32 * (i + 1), s0 * SEG : (s0 + HS) * SEG],
                    )

            acc = bpool.tile([128, KW], mybir.dt.float32, tag="acc")
            nc.vector.memset(acc[:, :], 0.0)

            def rv(tile, stride, cnt, off, width):
                """Strided-row view: rows of `width` elems at `off + i*stride`."""
                base = (off // stride) * stride
                o2 = off - base
                v = tile[:, base : base + cnt * stride].rearrange(
                    "p (s k) -> p s k", k=stride
                )
                return v[:, :, o2 : o2 + width]

            for w0, wc in CHUNKS:
                kc = kpool.tile([128, wc * KW], mybir.dt.bfloat16, tag="kc")
                nc.scalar.dma_start(out=kc[:, :], in_=keys2[:, w0 * KW : (w0 + wc) * KW])

                ne1 = wc + 4
                e1 = wpool.tile([128, ne1 * W_E1], mybir.dt.bfloat16, tag="e1")
                nc.vector.tensor_mul(
                    rv(e1, W_E1, ne1, 0, W_E1),
                    rv(g, SEG, ne1, (w0 + 1) * SEG, W_E1),
                    rv(g, SEG, ne1, (w0 + 3) * SEG + 2, W_E1),
                )
                e2 = wpool.tile([128, wc * KW], mybir.dt.bfloat16, tag="e2")
                nc.vector.tensor_mul(
                    rv(e2, KW, wc, 0, W_E2),
                    rv(e1, W_E1, wc, 0, W_E2),
                    rv(e1, W_E1, wc, 4 * W_E1 + 4, W_E2),
                )
                o3 = wpool.tile([128, wc * KW], mybir.dt.bfloat16, tag="o3")
                nc.vector.tensor_mul(
                    rv(o3, KW, wc, 0, W_O3),
                    rv(e1, W_E1, wc, W_E1, W_O3),
                    rv(g, SEG, wc, (w0 + 6) * SEG + 4, W_O3),
                )
                pr = wpool.tile([128, wc * KW], mybir.dt.bfloat16, tag="pr")
                nc.vector.tensor_mul(
                    rv(pr, KW, wc, 0, W_PR),
                    rv(e2, KW, wc, 0, W_PR),
                    rv(o3, KW, wc, 1, W_PR),
                )
                bd = wpool.tile([128, wc * KW], mybir.dt.bfloat16, tag="bd")
                nc.vector.tensor_mul(
                    rv(bd, KW, wc, 0, W_PR),
                    rv(pr, KW, wc, 0, W_PR),
                    rv(kc, KW, wc, 0, W_PR),
                )
                n = wc
                while n > 1:
                    m = n // 2
                    nc.vector.tensor_add(
                        rv(bd, KW, m, 0, W_PR),
                        rv(bd, KW, m, 0, W_PR),
                        rv(bd, KW, m, (n - m) * KW, W_PR),
                    )
                    n -= m
                nc.vector.tensor_add(acc[:, 0:W_PR], acc[:, 0:W_PR], bd[:, 0:W_PR])

            nc.sync.dma_start(out=out_r[:, :], in_=acc[:, 0:BLKW])
    nc.compile()
    return nc


def _host_prep(x, keys_weight, motion_table, hr_table):
    import ml_dtypes

    bf16 = ml_dtypes.bfloat16

    x0 = np.asarray(x[:, 0, :], dtype=np.float32)  # [B, F]
    # np.round and jnp.round both use round-half-to-even
    mi = np.round((x0[:, : F - 1] - (-3.0)) / (3.0 - (-3.0)) * (MROWS - 1)).astype(
        np.int32
    )
    mi = np.clip(mi, 0, MROWS - 1)
    hi = np.round((x0[:, F - 1] - 50.0) / (200.0 - 50.0) * (HROWS - 1)).astype(
        np.int32
    )
    hi = np.clip(hi, 0, HROWS - 1) + MROWS
    rows = np.concatenate([mi, hi[:, None]], axis=1).astype(np.int32)  # [B, F]

    if "tbl" not in _CACHE:
        tb = np.concatenate(
            [np.asarray(motion_table), np.asarray(hr_table)], axis=0
        ).astype(bf16)  # [VROWS, D]
        # Pre-blocked with halo: tbl[r, k*640 + e] = tb[r, (k*625 + e - 6) % D]
        # (cols 632..639 of each block are padding, never read by compute)
        col_idx = (
            np.arange(NBLK)[:, None] * BLKW + np.arange(SEG)[None, :] - 6
        ) % D  # [16, 640]
        _CACHE["tbl"] = np.ascontiguousarray(tb[:, col_idx.reshape(-1)])

        kb = np.asarray(keys_weight)[:W].astype(bf16)  # [57, D]
        karr = np.zeros((NBLK, W, KW), dtype=bf16)
        karr[:, :, :BLKW] = kb.reshape(W, NBLK, BLKW).transpose(1, 0, 2)
        _CACHE["keys2"] = np.tile(karr.reshape(NBLK, W * KW), (BPC, 1))
    tbl = _CACHE["tbl"]
    keys2 = _CACHE["keys2"]

    in_maps = []
    for c in range(NCORES):
        r8 = rows[BPC * c : BPC * (c + 1)]  # [8, F]
        # goff[p, i] = row index for batch 2i + p//64, pos p%64
        gof = r8.reshape(4, 2 * F).T.copy().astype(np.int32)  # [128, 4]
        in_maps.append({"tbl": tbl, "keys2": keys2, "goff": gof})
    return in_maps


def run(inputs, trace=False):
    if "nc" not in _CACHE:
        _CACHE["nc"] = _build_nc()
    nc = _CACHE["nc"]
    in_maps = _host_prep(**inputs)
    res = run_bass_kernel_spmd(nc, in_maps, core_ids=list(range(NCORES)), trace=trace)
    outs = [res.results[c]["out"] for c in range(NCORES)]
    full = np.concatenate(outs, axis=0).astype(np.float32)
    return full, res


def kernel(**inputs) -> np.ndarray:
    full, _ = run(inputs, trace=False)
    return full


# revision 21
# speedup vs baseline: 1.3447x; 1.0417x over previous
"""HDC generic encoder kernel for 8 Trainium2 NeuronCores.

out[b,d] = sum_{w=0..56} K[w,d] * prod_{j=0..6} enc0[b, w+1+j, (d-(6-j)) mod D]

enc0[b,p,:] is a +/-1 table row selected by level-quantizing x[b,0,p].
Sharding: pure data parallel over batch, 8 batches per core.

Device pipeline per core:
1. Table is pre-blocked in HBM as [VROWS, 16*632]: tbl[r, k*632+e] =
   row_r[(k*625 + e - 6) mod 10000], so each gathered row already holds
   the 16 d-blocks with their 7-element halos materialized.
2. 4 indirect row-gathers R_i[128, 10112] for batches {2i, 2i+1}
   (partition p = h*64 + pos -> batch 2i+h).
3. Redistribution to compute layout G[q = b*16 + blk][s*632 + e] via a
   DRAM bounce: R_i -> dram scratch in G-order (src reads 128 spread
   partitions, dst scatters 1264B runs in DRAM - both fast DMA
   patterns), then a contiguous 80KB-per-partition readback into G.
   This replaces the direct SBUF->SBUF transpose whose 64 packets per
   DMA all targeted ONE dst partition (one port) and serialized at
   ~737 ns/packet (~380 us total in the previous version).
4. Window products via a 5-op tensor_tensor tree (even offsets
   everywhere except the final even*odd combine which runs at 1x):
     E1[p,e]  = G[p*SEG+e]   * G[(p+2)*SEG+e+2]     (factor pairs j, j+2)
     E2[w,d'] = E1[w+1,d']   * E1[w+5,d'+4]         (factors 0,2,4,6)
     O3[w,e]  = E1[w+2,e]    * G[(w+6)*SEG+e+4]     (factors 1,3,5; e=d'+1)
     PROD     = E2[w,d']     * O3[w,d'+1]
     BD       = PROD * keys
5. log-tree adds over w (exact in bf16: +-1 sums <= 57), f32 accumulation
   across w-chunks, one DMA of the [128, 625] accumulator to out [8, 10000].
"""

import numpy as np

import concourse.bacc as bacc
import concourse.bass as bass
import concourse.mybir as mybir
from concourse.bass_utils import run_bass_kernel_spmd
from concourse.tile import TileContext

B, T, F, D = 64, 4, 64, 10000
NGRAMS = 7
W = F - NGRAMS  # 57 windows
NCORES = 8
BPC = B // NCORES  # 8 batches per core
MROWS, HROWS = 3000, 200
VROWS = MROWS + HROWS

NBLK = 16
BLKW = D // NBLK  # 625
SEGW = 632  # valid elems per segment (625 + 6 halo + 1)
SEG = 640  # segment stride in G (640 not 632: 632 measured ~27% slower DVE)
ET = NBLK * SEG  # pre-blocked table row width: 10240 (blocks padded to 640)
KW = 626  # keys/intermediate per-window stride (even)
SGRP = 8  # segments per bounce/readback group (pipelining granularity)

W_E1 = 630
W_E2 = 626
W_O3 = 626
W_PR = 625

CHUNKS = [(0, 8), (8, 8), (16, 8), (24, 8), (32, 8), (40, 8), (48, 9)]

_CACHE = {}


def _build_nc():
    nc = bacc.Bacc(None)
    tbl = nc.dram_tensor("tbl", [VROWS, ET], mybir.dt.bfloat16, kind="ExternalInput")
    keys2 = nc.dram_tensor(
        "keys2", [128, W * KW], mybir.dt.bfloat16, kind="ExternalInput"
    )
    goff = nc.dram_tensor("goff", [128, 4], mybir.dt.int32, kind="ExternalInput")
    out = nc.dram_tensor("out", [BPC, D], mybir.dt.float32, kind="ExternalOutput")
    out_r = out.rearrange("b (q d) -> (b q) d", d=BLKW)  # [128, 625]

    gb = nc.dram_tensor("gbounce", [128, F * SEG], mybir.dt.bfloat16, kind="Internal")
    # gbv[b, s, k, e]: G-order view of the bounce buffer (row q = 16b + k).
    # Same 640-stride layout as G so the readback is one contiguous copy.
    gbv = gb.rearrange("(b k) (s e) -> b s k e", k=NBLK, e=SEG)

    with TileContext(nc) as tc:
        with (
            tc.tile_pool(name="big", bufs=1) as bpool,
            tc.tile_pool(name="rp", bufs=3) as rpool,
            tc.tile_pool(name="work", bufs=1) as wpool,
            tc.tile_pool(name="keysp", bufs=2) as kpool,
        ):
            goff_t = bpool.tile([128, 4], mybir.dt.int32, tag="goff")
            nc.sync.dma_start(out=goff_t[:, :], in_=goff[:, :])

            g = bpool.tile([128, F * SEG], mybir.dt.bfloat16, tag="G")
            # bounce on the SP ring (nc.sync), readback on the ACT ring
            # (nc.scalar) so readback semaphore waits never stall the SP
            # sequencer.  Readbacks are quartered by segment range, with the
            # first quarter of every tile issued first, so compute on early
            # window chunks starts as soon as segments 0..15 have landed.
            QS = F // 4  # 16 segments per readback quarter
            for i in range(4):
                r = rpool.tile([128, ET], mybir.dt.bfloat16, tag="R", name=f"R{i}")
                nc.gpsimd.indirect_dma_start(
                    out=r[:, :],
                    out_offset=None,
                    in_=tbl[:, :],
                    in_offset=bass.IndirectOffsetOnAxis(ap=goff_t[:, i : i + 1], axis=0),
                )
                r3 = r[:, :].rearrange("p (k e) -> p k e", e=SEG)
                for h in range(2):
                    nc.sync.dma_start(
                        out=gbv[2 * i + h, :, :, :],
                        in_=r3[64 * h : 64 * (h + 1), :, :],
                    )
                nc.scalar.dma_start(
                    out=g[32 * i : 32 * (i + 1), 0 : QS * SEG],
                    in_=gb[32 * i : 32 * (i + 1), 0 : QS * SEG],
                )
            for q in range(1, 4):
                for i in range(4):
                    nc.scalar.dma_start(
                        out=g[32 * i : 32 * (i + 1), q * QS * SEG : (q + 1) * QS * SEG],
                        in_=gb[32 * i : 32 * (i + 1), q * QS * SEG : (q + 1) * QS * SEG],
                    )

            acc = bpool.tile([128, KW], mybir.dt.float32, tag="acc")
            nc.vector.memset(acc[:, :], 0.0)

            def rv(tile, stride, cnt, off, width):
                """Strided-row view: rows of `width` elems at `off + i*stride`."""
                base = (off // stride) * stride
                o2 = off - base
                v = tile[:, base : base + cnt * stride].rearrange(
                    "p (s k) -> p s k", k=stride
                )
                return v[:, :, o2 : o2 + width]

            for w0, wc in CHUNKS:
                kc = kpool.tile([128, wc * KW], mybir.dt.bfloat16, tag="kc")
                nc.scalar.dma_start(out=kc[:, :], in_=keys2[:, w0 * KW : (w0 + wc) * KW])

                ne1 = wc + 4
                e1 = wpool.tile([128, ne1 * W_E1], mybir.dt.bfloat16, tag="e1")
                nc.vector.tensor_mul(
                    rv(e1, W_E1, ne1, 0, W_E1),
                    rv(g, SEG, ne1, (w0 + 1) * SEG, W_E1),
                    rv(g, SEG, ne1, (w0 + 3) * SEG + 2, W_E1),
                )
                e2 = wpool.tile([128, wc * KW], mybir.dt.bfloat16, tag="e2")
                nc.vector.tensor_mul(
                    rv(e2, KW, wc, 0, W_E2),
                    rv(e1, W_E1, wc, 0, W_E2),
                    rv(e1, W_E1, wc, 4 * W_E1 + 4, W_E2),
                )
                o3 = wpool.tile([128, wc * KW], mybir.dt.bfloat16, tag="o3")
                nc.vector.tensor_mul(
                    rv(o3, KW, wc, 0, W_O3),
                    rv(e1, W_E1, wc, W_E1, W_O3),
                    rv(g, SEG, wc, (w0 + 6) * SEG + 4, W_O3),
                )
                # pr reuses e1's buffer (e1 fully consumed by e2/o3), and bd
                # reuses e2's (consumed by pr) - Tile's WAR tracking orders it.
                pr = e1
                nc.vector.tensor_mul(
                    rv(pr, KW, wc, 0, W_PR),
                    rv(e2, KW, wc, 0, W_PR),
                    rv(o3, KW, wc, 1, W_PR),
                )
                bd = e2
                nc.vector.tensor_mul(
                    rv(bd, KW, wc, 0, W_PR),
                    rv(pr, KW, wc, 0, W_PR),
                    rv(kc, KW, wc, 0, W_PR),
                )
                n = wc
                while n > 1:
                    m = n // 2
                    nc.vector.tensor_add(
                        rv(bd, KW, m, 0, W_PR),
                        rv(bd, KW, m, 0, W_PR),
                        rv(bd, KW, m, (n - m) * KW, W_PR),
                    )
                    n -= m
                nc.vector.tensor_add(acc[:, 0:W_PR], acc[:, 0:W_PR], bd[:, 0:W_PR])

            nc.sync.dma_start(out=out_r[:, :], in_=acc[:, 0:BLKW])
    nc.compile()
    return nc


def _host_prep(x, keys_weight, motion_table, hr_table):
    import ml_dtypes

    bf16 = ml_dtypes.bfloat16

    x0 = np.asarray(x[:, 0, :], dtype=np.float32)  # [B, F]
    # np.round and jnp.round both use round-half-to-even
    mi = np.round((x0[:, : F - 1] - (-3.0)) / (3.0 - (-3.0)) * (MROWS - 1)).astype(
        np.int32
    )
    mi = np.clip(mi, 0, MROWS - 1)
    hi = np.round((x0[:, F - 1] - 50.0) / (200.0 - 50.0) * (HROWS - 1)).astype(
        np.int32
    )
    hi = np.clip(hi, 0, HROWS - 1) + MROWS
    rows = np.concatenate([mi, hi[:, None]], axis=1).astype(np.int32)  # [B, F]

    if "tbl" not in _CACHE:
        tb = np.concatenate(
            [np.asarray(motion_table), np.asarray(hr_table)], axis=0
        ).astype(bf16)  # [VROWS, D]
        # Pre-blocked with halo: tbl[r, k*640 + e] = tb[r, (k*625 + e - 6) % D]
        # (cols 632..639 of each block are padding, never read by compute)
        col_idx = (
            np.arange(NBLK)[:, None] * BLKW + np.arange(SEG)[None, :] - 6
        ) % D  # [16, 640]
        _CACHE["tbl"] = np.ascontiguousarray(tb[:, col_idx.reshape(-1)])

        kb = np.asarray(keys_weight)[:W].astype(bf16)  # [57, D]
        karr = np.zeros((NBLK, W, KW), dtype=bf16)
        karr[:, :, :BLKW] = kb.reshape(W, NBLK, BLKW).transpose(1, 0, 2)
        _CACHE["keys2"] = np.tile(karr.reshape(NBLK, W * KW), (BPC, 1))
    tbl = _CACHE["tbl"]
    keys2 = _CACHE["keys2"]

    in_maps = []
    for c in range(NCORES):
        r8 = rows[BPC * c : BPC * (c + 1)]  # [8, F]
        # goff[p, i] = row index for batch 2i + p//64, pos p%64
        gof = r8.reshape(4, 2 * F).T.copy().astype(np.int32)  # [128, 4]
        in_maps.append({"tbl": tbl, "keys2": keys2, "goff": gof})
    return in_maps


def run(inputs, trace=False):
    if "nc" not in _CACHE:
        _CACHE["nc"] = _build_nc()
    nc = _CACHE["nc"]
    in_maps = _host_prep(**inputs)
    res = run_bass_kernel_spmd(nc, in_maps, core_ids=list(range(NCORES)), trace=trace)
    outs = [res.results[c]["out"] for c in range(NCORES)]
    full = np.concatenate(outs, axis=0).astype(np.float32)
    return full, res


def kernel(**inputs) -> np.ndarray:
    full, _ = run(inputs, trace=False)
    return full


# revision 22
# speedup vs baseline: 1.6248x; 1.2083x over previous
"""HDC generic encoder kernel for 8 Trainium2 NeuronCores.

out[b,d] = sum_{w=0..56} K[w,d] * prod_{j=0..6} enc0[b, w+1+j, (d-(6-j)) mod D]

enc0[b,p,:] is a +/-1 table row selected by level-quantizing x[b,0,p].
Sharding: pure data parallel over batch, 8 batches per core.

Device pipeline per core:
1. Table is pre-blocked in HBM as [VROWS, 16*632]: tbl[r, k*632+e] =
   row_r[(k*625 + e - 6) mod 10000], so each gathered row already holds
   the 16 d-blocks with their 7-element halos materialized.
2. 4 indirect row-gathers R_i[128, 10112] for batches {2i, 2i+1}
   (partition p = h*64 + pos -> batch 2i+h).
3. Redistribution to compute layout G[q = b*16 + blk][s*632 + e] via a
   DRAM bounce: R_i -> dram scratch in G-order (src reads 128 spread
   partitions, dst scatters 1264B runs in DRAM - both fast DMA
   patterns), then a contiguous 80KB-per-partition readback into G.
   This replaces the direct SBUF->SBUF transpose whose 64 packets per
   DMA all targeted ONE dst partition (one port) and serialized at
   ~737 ns/packet (~380 us total in the previous version).
4. Window products via a 5-op tensor_tensor tree (even offsets
   everywhere except the final even*odd combine which runs at 1x):
     E1[p,e]  = G[p*SEG+e]   * G[(p+2)*SEG+e+2]     (factor pairs j, j+2)
     E2[w,d'] = E1[w+1,d']   * E1[w+5,d'+4]         (factors 0,2,4,6)
     O3[w,e]  = E1[w+2,e]    * G[(w+6)*SEG+e+4]     (factors 1,3,5; e=d'+1)
     PROD     = E2[w,d']     * O3[w,d'+1]
     BD       = PROD * keys
5. log-tree adds over w (exact in bf16: +-1 sums <= 57), f32 accumulation
   across w-chunks, one DMA of the [128, 625] accumulator to out [8, 10000].
"""

import numpy as np

import concourse.bacc as bacc
import concourse.bass as bass
import concourse.mybir as mybir
from concourse.bass_utils import run_bass_kernel_spmd
from concourse.tile import TileContext

B, T, F, D = 64, 4, 64, 10000
NGRAMS = 7
W = F - NGRAMS  # 57 windows
NCORES = 8
BPC = B // NCORES  # 8 batches per core
MROWS, HROWS = 3000, 200
VROWS = MROWS + HROWS

NBLK = 16
BLKW = D // NBLK  # 625
SEGW = 632  # valid elems per segment (625 + 6 halo + 1)
SEG = 640  # segment stride in G (640 not 632: 632 measured ~27% slower DVE)
ET = NBLK * SEG  # pre-blocked table row width: 10240 (blocks padded to 640)
KW = 626  # keys/intermediate per-window stride (even)
SGRP = 8  # segments per bounce/readback group (pipelining granularity)

W_E1 = 630
W_E2 = 626
W_O3 = 626
W_PR = 625

CHUNKS = [(0, 8), (8, 8), (16, 8), (24, 8), (32, 8), (40, 8), (48, 9)]

_CACHE = {}


def _build_nc():
    nc = bacc.Bacc(None)
    tbl = nc.dram_tensor("tbl", [VROWS, ET], mybir.dt.float8e4, kind="ExternalInput")
    keys2 = nc.dram_tensor(
        "keys2", [128, W * KW], mybir.dt.bfloat16, kind="ExternalInput"
    )
    goff = nc.dram_tensor("goff", [128, 4], mybir.dt.int32, kind="ExternalInput")
    out = nc.dram_tensor("out", [BPC, D], mybir.dt.float32, kind="ExternalOutput")
    out_r = out.rearrange("b (q d) -> (b q) d", d=BLKW)  # [128, 625]

    gb = nc.dram_tensor("gbounce", [128, F * SEG], mybir.dt.float8e4, kind="Internal")
    # gbv[b, s, k, e]: G-order view of the bounce buffer (row q = 16b + k).
    # Same 640-stride layout as G so the readback is one contiguous copy.
    gbv = gb.rearrange("(b k) (s e) -> b s k e", k=NBLK, e=SEG)

    with TileContext(nc) as tc:
        with (
            tc.tile_pool(name="big", bufs=1) as bpool,
            tc.tile_pool(name="rp", bufs=4) as rpool,
            tc.tile_pool(name="work", bufs=1) as wpool,
            tc.tile_pool(name="keysp", bufs=2) as kpool,
        ):
            goff_t = bpool.tile([128, 4], mybir.dt.int32, tag="goff")
            nc.sync.dma_start(out=goff_t[:, :], in_=goff[:, :])

            g = bpool.tile([128, F * SEG], mybir.dt.bfloat16, tag="G")
            # bounce on the SP ring (nc.sync), readback on the ACT ring
            # (nc.scalar) so readback semaphore waits never stall the SP
            # sequencer.  Readbacks are quartered by segment range, with the
            # first quarter of every tile issued first, so compute on early
            # window chunks starts as soon as segments 0..15 have landed.
            QS = F // 4  # 16 segments per readback quarter
            for i in range(4):
                r = rpool.tile([128, ET], mybir.dt.float8e4, tag="R", name=f"R{i}")
                nc.gpsimd.indirect_dma_start(
                    out=r[:, :],
                    out_offset=None,
                    in_=tbl[:, :],
                    in_offset=bass.IndirectOffsetOnAxis(ap=goff_t[:, i : i + 1], axis=0),
                )
                r3 = r[:, :].rearrange("p (k e) -> p k e", e=SEG)
                for h in range(2):
                    nc.sync.dma_start(
                        out=gbv[2 * i + h, :, :, :],
                        in_=r3[64 * h : 64 * (h + 1), :, :],
                    )
                nc.gpsimd.dma_start(
                    out=g[32 * i : 32 * (i + 1), 0 : QS * SEG],
                    in_=gb[32 * i : 32 * (i + 1), 0 : QS * SEG],
                )
            for q in range(1, 4):
                for i in range(4):
                    nc.gpsimd.dma_start(
                        out=g[32 * i : 32 * (i + 1), q * QS * SEG : (q + 1) * QS * SEG],
                        in_=gb[32 * i : 32 * (i + 1), q * QS * SEG : (q + 1) * QS * SEG],
                    )

            acc = bpool.tile([128, KW], mybir.dt.float32, tag="acc")
            nc.vector.memset(acc[:, :], 0.0)

            def rv(tile, stride, cnt, off, width):
                """Strided-row view: rows of `width` elems at `off + i*stride`."""
                base = (off // stride) * stride
                o2 = off - base
                v = tile[:, base : base + cnt * stride].rearrange(
                    "p (s k) -> p s k", k=stride
                )
                return v[:, :, o2 : o2 + width]

            for w0, wc in CHUNKS:
                kc = kpool.tile([128, wc * KW], mybir.dt.bfloat16, tag="kc")
                nc.scalar.dma_start(out=kc[:, :], in_=keys2[:, w0 * KW : (w0 + wc) * KW])

                ne1 = wc + 4
                e1 = wpool.tile([128, ne1 * W_E1], mybir.dt.bfloat16, tag="e1")
                nc.vector.tensor_mul(
                    rv(e1, W_E1, ne1, 0, W_E1),
                    rv(g, SEG, ne1, (w0 + 1) * SEG, W_E1),
                    rv(g, SEG, ne1, (w0 + 3) * SEG + 2, W_E1),
                )
                e2 = wpool.tile([128, wc * KW], mybir.dt.bfloat16, tag="e2")
                nc.vector.tensor_mul(
                    rv(e2, KW, wc, 0, W_E2),
                    rv(e1, W_E1, wc, 0, W_E2),
                    rv(e1, W_E1, wc, 4 * W_E1 + 4, W_E2),
                )
                o3 = wpool.tile([128, wc * KW], mybir.dt.bfloat16, tag="o3")
                nc.vector.tensor_mul(
                    rv(o3, KW, wc, 0, W_O3),
                    rv(e1, W_E1, wc, W_E1, W_O3),
                    rv(g, SEG, wc, (w0 + 6) * SEG + 4, W_O3),
                )
                # pr reuses e1's buffer (e1 fully consumed by e2/o3), and bd
                # reuses e2's (consumed by pr) - Tile's WAR tracking orders it.
                pr = e1
                nc.vector.tensor_mul(
                    rv(pr, KW, wc, 0, W_PR),
                    rv(e2, KW, wc, 0, W_PR),
                    rv(o3, KW, wc, 1, W_PR),
                )
                bd = e2
                nc.vector.tensor_mul(
                    rv(bd, KW, wc, 0, W_PR),
                    rv(pr, KW, wc, 0, W_PR),
                    rv(kc, KW, wc, 0, W_PR),
                )
                n = wc
                while n > 1:
                    m = n // 2
                    nc.vector.tensor_add(
                        rv(bd, KW, m, 0, W_PR),
                        rv(bd, KW, m, 0, W_PR),
                        rv(bd, KW, m, (n - m) * KW, W_PR),
                    )
                    n -= m
                nc.vector.tensor_add(acc[:, 0:W_PR], acc[:, 0:W_PR], bd[:, 0:W_PR])

            nc.sync.dma_start(out=out_r[:, :], in_=acc[:, 0:BLKW])
    nc.compile()
    return nc


def _host_prep(x, keys_weight, motion_table, hr_table):
    import ml_dtypes

    bf16 = ml_dtypes.bfloat16

    x0 = np.asarray(x[:, 0, :], dtype=np.float32)  # [B, F]
    # np.round and jnp.round both use round-half-to-even
    mi = np.round((x0[:, : F - 1] - (-3.0)) / (3.0 - (-3.0)) * (MROWS - 1)).astype(
        np.int32
    )
    mi = np.clip(mi, 0, MROWS - 1)
    hi = np.round((x0[:, F - 1] - 50.0) / (200.0 - 50.0) * (HROWS - 1)).astype(
        np.int32
    )
    hi = np.clip(hi, 0, HROWS - 1) + MROWS
    rows = np.concatenate([mi, hi[:, None]], axis=1).astype(np.int32)  # [B, F]

    if "tbl" not in _CACHE:
        fp8 = __import__("concourse.mybir", fromlist=["dt"]).dt.np(
            __import__("concourse.mybir", fromlist=["dt"]).dt.float8e4
        )
        tb = np.concatenate(
            [np.asarray(motion_table), np.asarray(hr_table)], axis=0
        ).astype(fp8)  # [VROWS, D]
        # Pre-blocked with halo: tbl[r, k*640 + e] = tb[r, (k*625 + e - 6) % D]
        # (cols 632..639 of each block are padding, never read by compute)
        col_idx = (
            np.arange(NBLK)[:, None] * BLKW + np.arange(SEG)[None, :] - 6
        ) % D  # [16, 640]
        _CACHE["tbl"] = np.ascontiguousarray(tb[:, col_idx.reshape(-1)])

        kb = np.asarray(keys_weight)[:W].astype(bf16)  # [57, D]
        karr = np.zeros((NBLK, W, KW), dtype=bf16)
        karr[:, :, :BLKW] = kb.reshape(W, NBLK, BLKW).transpose(1, 0, 2)
        _CACHE["keys2"] = np.tile(karr.reshape(NBLK, W * KW), (BPC, 1))
    tbl = _CACHE["tbl"]
    keys2 = _CACHE["keys2"]

    in_maps = []
    for c in range(NCORES):
        r8 = rows[BPC * c : BPC * (c + 1)]  # [8, F]
        # goff[p, i] = row index for batch 2i + p//64, pos p%64
        gof = r8.reshape(4, 2 * F).T.copy().astype(np.int32)  # [128, 4]
        in_maps.append({"tbl": tbl, "keys2": keys2, "goff": gof})
    return in_maps


def run(inputs, trace=False):
    if "nc" not in _CACHE:
        _CACHE["nc"] = _build_nc()
    nc = _CACHE["nc"]
    in_maps = _host_prep(**inputs)
    res = run_bass_kernel_spmd(nc, in_maps, core_ids=list(range(NCORES)), trace=trace)
    outs = [res.results[c]["out"] for c in range(NCORES)]
    full = np.concatenate(outs, axis=0).astype(np.float32)
    return full, res


def kernel(**inputs) -> np.ndarray:
    full, _ = run(inputs, trace=False)
    return full
